# revision 31
# baseline (speedup 1.0000x reference)
"""Trainium2 Bass kernel for nn_CA_Module (channel-attention + SE gating).

Reference computation per sample (C=512, N=H*W=4096):
    q = x.reshape(C, N)
    energy = q @ q.T                     # [C, C]
    att = softmax(max_row - energy)      # == softmax(-energy)  (row shift cancels)
        -> G = exp(min_row - energy); att = G / rowsum(G)
    out = att @ q                        # [C, N]
    pooled = concat([mean_n(x), mean_n(out)])        # [2C]
    h  = relu(w1 @ pooled + b1)                      # [64]
    se = sigmoid(w2 @ h + b2)                        # [C]
    y  = se * x + (1 - se) * out

Structure (one NeuronCore, 2 samples):
  * both samples' x DMA-in is issued up front (1 MB chunks) so sample 1's
    load is never queued behind sample 0's output DMA in the HWDGE FIFO.
  * phase A (per 128-col n-slice): PE transposes q (bf16 identity operand
    -> 1 cyc/row), DVE/ACT alternate on the PSUM->SBUF evacuation, PE
    accumulates the upper-triangular energy blocks in fp32r.  Row-block 3
    is computed at free-dim 256 (redundant lower half) because fp32r
    matmuls below 256 free-dim run at 1/4 rate; the extra block is the
    (3,2) mirror for free.
  * pooled-x runs on the otherwise idle GpSimd engine.
  * phase B: mirror the lower-triangular energy blocks (PE transpose +
    DVE copy), row-min (DVE), exp in-place (ACT, accumulating rowsum S).
  * phase C: G^T staged bank-by-bank (PE transpose + DVE copy), then the
    tiny pooled-out / SE-gate matvecs.
  * phase D: out = G @ q with stationary reuse in 3/3/2-chunk PSUM bank
    groups; ACT applies beta=(1-se)/S (bf16), DVE fuses the blend
    y = se*x + beta*(G@q) and writes bf16; y is DMA'd out as bf16 (host
    converts back to fp32; the 2e-2 rel-err budget dwarfs bf16 rounding).

Sharding: data-parallel over batch, 2 samples per core on 8 cores.
"""

import numpy as np

try:
    import concourse.bass as bass
except ImportError:
    import sys

    sys.path.insert(0, "/opt/trn_rl_repo")
    import concourse.bass as bass

import concourse.tile as tile
from concourse import bacc, mybir
from concourse import bass_utils as _bu
from concourse.bass_utils import run_bass_kernel_spmd
from concourse.masks import make_identity

# Enable walrus's weight-load optimization (background-buffer LDW overlap /
# dedup). The concourse default passes --enable-ldw-opt=false; measured on
# hardware this costs ~2x on 4-byte matmul streams, and enabling it is
# numerically verified on this kernel.
if not getattr(_bu, "_ldw_opt_patched", False):
    _orig_run_command = _bu.run_command

    def _run_command_ldw(cmd, *a, **k):
        if isinstance(cmd, list):
            cmd = [
                "--enable-ldw-opt=true" if c == "--enable-ldw-opt=false" else c
                for c in cmd
            ]
        return _orig_run_command(cmd, *a, **k)

    _bu.run_command = _run_command_ldw
    _bu._ldw_opt_patched = True

F32 = mybir.dt.float32
F32R = mybir.dt.float32r
BF16 = mybir.dt.bfloat16
AF = mybir.ActivationFunctionType
ALU = mybir.AluOpType
AX = mybir.AxisListType

B_TOTAL = 16
N_CORES = 8
B_PER_CORE = B_TOTAL // N_CORES  # 2
C = 512
N = 4096
CB = C // 128  # 4 c-blocks
KT = N // 128  # 32 n-slices for transpose/mm1
# mm1 free-dim per row block (block 3 padded to 256: fp32r needs fd>=256)
MM1_FD = [512, 384, 256, 256]
MM1_D0 = [0, 128, 256, 256]  # first d-column of each row block's output


def _build_program(reps: int = 1, hw_loop: bool = False) -> bass.Bass:
    nc = bacc.Bacc(target_bir_lowering=False, debug=False)

    x_d = nc.dram_tensor("x", [B_PER_CORE, C, N], F32, kind="ExternalInput").ap()
    w1_d = nc.dram_tensor("w1", [64, 2 * C], F32, kind="ExternalInput").ap()
    b1_d = nc.dram_tensor("b1", [64, 1], F32, kind="ExternalInput").ap()
    w2_d = nc.dram_tensor("w2", [C, 64], F32, kind="ExternalInput").ap()
    b2_d = nc.dram_tensor("b2", [C, 1], F32, kind="ExternalInput").ap()
    y_d = nc.dram_tensor("y", [B_PER_CORE, C, N], BF16, kind="ExternalOutput").ap()

    with tile.TileContext(nc) as tc:
        _emit(tc, x_d, w1_d, b1_d, w2_d, b2_d, y_d, reps, hw_loop)
    nc.compile()
    return nc


def _emit(tc, x_d, w1_d, b1_d, w2_d, b2_d, y_d, reps=1, hw_loop=False):
    nc = tc.nc
    from contextlib import ExitStack

    with ExitStack() as ctx:
        singles = ctx.enter_context(tc.tile_pool(name="singles", bufs=1))
        qpool = ctx.enter_context(tc.tile_pool(name="qpool", bufs=2))
        qtpool = ctx.enter_context(tc.tile_pool(name="qtpool", bufs=5))
        gpool = ctx.enter_context(tc.tile_pool(name="gpool", bufs=1))
        gtpool = ctx.enter_context(tc.tile_pool(name="gtpool", bufs=2))
        stats = ctx.enter_context(tc.tile_pool(name="stats", bufs=2))
        outp = ctx.enter_context(tc.tile_pool(name="outp", bufs=2))
        psum = ctx.enter_context(tc.tile_pool(name="psum", bufs=1, space="PSUM"))

        # rep 0's sample-0 x stream leads the HWDGE FIFO; the weight/bias
        # DMAs ride between the two samples' streams (they are only needed
        # by the late SE-gate matvecs), so phase A is never DMA-starved by
        # descriptor-generation serialization.
        qs0 = [
            qpool.tile([128, CB, N], F32R, tag="q", name=f"q_r0_{b}")
            for b in range(B_PER_CORE)
        ]
        for lo, hi in ((0, 512), (512, 1024), (1024, 2048), (2048, 3072), (3072, 4096)):
            for m in range(CB):
                nc.sync.dma_start(
                    out=qs0[0][:, m, lo:hi],
                    in_=x_d[0, 128 * m : 128 * (m + 1), lo:hi].bitcast(F32R),
                )

        # ---- one-time setup (no DMA dependencies) ---------------------------
        ident = singles.tile([128, 128], F32)
        make_identity(nc, ident)
        ident_r = singles.tile([128, 128], F32R)
        nc.vector.tensor_copy(ident_r, ident)
        # warm-up transposes: absorb identity-producer waits into the PE clock
        warm = psum.tile([128, 128], F32, tag="tstage", bufs=3)
        nc.tensor.transpose(warm, ident, ident)
        warm2 = psum.tile([128, 128], F32, tag="tstage", bufs=3)
        nc.tensor.transpose(warm2.bitcast(F32R), ident_r, ident_r)

        # weight/bias loads, then rep 0's sample-1 x stream
        w1_nat = singles.tile([64, 2 * C], F32)
        nc.sync.dma_start(out=w1_nat, in_=w1_d)
        w2_nat = singles.tile([128, CB, 64], F32)
        for m in range(CB):
            nc.sync.dma_start(
                out=w2_nat[:, m, :], in_=w2_d[128 * m : 128 * (m + 1), :]
            )
        b1_t = singles.tile([64, 1], F32)
        nc.sync.dma_start(out=b1_t, in_=b1_d)
        b2_t = singles.tile([128, CB], F32)
        for m in range(CB):
            nc.sync.dma_start(out=b2_t[:, m : m + 1], in_=b2_d[128 * m : 128 * (m + 1), :])
        for h in range(4):
            nsl = slice(1024 * h, 1024 * (h + 1))
            for m in range(CB):
                nc.sync.dma_start(
                    out=qs0[1][:, m, nsl],
                    in_=x_d[1, 128 * m : 128 * (m + 1), nsl].bitcast(F32R),
                )

        # weight prep (PE transposes + copies) is emitted lazily -- inside
        # rep 0 right before sample 0's phase C -- so it sits in the PE queue
        # after phase A instead of blocking it while the w DMAs are in flight.
        w1T = singles.tile([128, 8, 64], F32)
        w2T = singles.tile([64, CB, 128], F32)
        b2h_t = singles.tile([128, CB], F32)
        prep_done = [False]

        def weight_prep():
            if prep_done[0]:
                return
            prep_done[0] = True
            for k in range(8):
                tp = psum.tile([128, 64], F32, tag="tstage", bufs=3)
                nc.tensor.transpose(
                    tp, w1_nat[0:64, 128 * k : 128 * (k + 1)], ident[0:64, 0:64]
                )
                nc.vector.tensor_copy(w1T[:, k, :], tp)
            for m in range(CB):
                tp = psum.tile([128, 128], F32, tag="tstage", bufs=3)
                nc.tensor.transpose(tp[0:64, :], w2_nat[:, m, :], ident)
                nc.vector.tensor_copy(w2T[:, m, :], tp[0:64, :])
            nc.scalar.mul(b2h_t, b2_t, 0.5)

        consts = (ident, ident_r, w1T, w2T, b1_t, b2h_t, weight_prep)

        if hw_loop and reps > 1:
            with tc.For_i(0, reps, 1):
                _emit_rep(tc, x_d, y_d, consts, qpool, qtpool, gpool, gtpool,
                          stats, outp, psum, 0, prefetch=False)
        else:
            for rep in range(reps):
                _emit_rep(tc, x_d, y_d, consts, qpool, qtpool, gpool, gtpool,
                          stats, outp, psum, rep, prefetch=(reps > 1),
                          qs=qs0 if rep == 0 else None)


def _emit_in_dmas(nc, x_d, qs, rep):
    """x load for both samples: 512KB chunks [128, 1024], window-major so the
    first 4 n-slices are ready after 2MB instead of 4MB."""
    for b in range(B_PER_CORE):
        windows = [(0, 512), (512, 1024)] + [
            (1024 * h, 1024 * (h + 1)) for h in range(1, 4)
        ] if b == 0 else [(1024 * h, 1024 * (h + 1)) for h in range(4)]
        for lo, hi in windows:
            for m in range(CB):
                nc.sync.dma_start(
                    out=qs[b][:, m, lo:hi],
                    in_=x_d[b, 128 * m : 128 * (m + 1), lo:hi].bitcast(F32R),
                )


def _emit_rep(tc, x_d, y_d, consts, qpool, qtpool, gpool, gtpool, stats, outp,
              psum, rep, prefetch=False, qs=None):
    """Software-pipelined emission: sample 1's phase-A units are interleaved
    into sample 0's B/C/D emission so the static per-engine instruction
    orders (which follow emission order) let the scheduler overlap them."""
    nc = tc.nc

    if qs is None:
        qs = [
            qpool.tile([128, CB, N], F32R, tag="q", name=f"q_r{rep}_{b}")
            for b in range(B_PER_CORE)
        ]
        _emit_in_dmas(nc, x_d, qs, rep)

    st = [dict(q=qs[b]) for b in range(B_PER_CORE)]
    A = [_phase_A_units(tc, st[b], consts, qtpool, stats, psum, rep, b)
         for b in range(B_PER_CORE)]
    D = []

    # sample 0 phase A
    for u in A[0]:
        u["te"]()
        u["mm"]()
    _emit_phase_B(tc, st[0], consts, gpool, stats, psum, rep, 0)
    _emit_pooled(tc, st[0], stats, rep, 0)
    # sample-1 transpose/evac slices fill sample 0's serial softmax chain;
    # their energy matmuls are deferred (they would block the strict PE
    # queue while sample 0 still holds the PSUM banks)
    for u in A[1][:4]:
        u["te"]()
    consts[-1]()  # weight prep: PE transposes queued after phase A
    _emit_phase_C(tc, st[0], consts, gtpool, stats, psum, rep, 0)
    for u in A[1][4:6]:
        u["te"]()
    D.append(_phase_D_groups(tc, st[0], y_d, stats, outp, psum, rep, 0))
    # interleave: sample 0 phase D groups with sample 1's A units
    pend_mm = [u["mm"] for u in A[1][:6]]
    rem = list(A[1][6:])
    gi = 0
    for g in D[0]:
        g()
        if pend_mm:
            for mm in pend_mm[:3]:
                mm()
            pend_mm = pend_mm[3:]
            continue
        take = rem[: 2 + (gi % 2)]
        rem = rem[len(take):]
        for u in take:
            u["te"]()
            u["mm"]()
        gi += 1
    for u in rem:
        u["te"]()
        u["mm"]()
    _emit_phase_B(tc, st[1], consts, gpool, stats, psum, rep, 1)
    _emit_pooled(tc, st[1], stats, rep, 1)
    _emit_phase_C(tc, st[1], consts, gtpool, stats, psum, rep, 1)
    for g in _phase_D_groups(tc, st[1], y_d, stats, outp, psum, rep, 1):
        g()


def _phase_A_units(tc, st, consts, qtpool, stats, psum, rep, b):
    """One closure per n-slice: 4 PE transposes, one PSUM->SBUF evacuation
    (DVE/ACT alternating), 4 fp32r energy matmuls."""
    nc = tc.nc
    ident, ident_r, w1T, w2T, b1_t, b2h_t, weight_prep = consts
    q = st["q"]
    # energy row-blocks packed 3-banks-per-sample:
    #   bank0: eps0 [0,512); bank1: eps1 [0,384); bank2: eps2+eps3 (256+256)
    eb0 = psum.tile([128, 512], F32, tag="bank", bufs=5, name=f"e0_{rep}_{b}")
    eb1 = psum.tile([128, 512], F32, tag="bank", bufs=5, name=f"e1_{rep}_{b}")
    eb2 = psum.tile([128, 512], F32, tag="bank", bufs=5, name=f"e2_{rep}_{b}")
    st["eps"] = [eb0, eb1[:, 0:384], eb2[:, 0:256], eb2[:, 256:512]]
    eps = st["eps"]
    st["px_part"] = stats.tile([128, CB, 2], F32, tag="pxp", name=f"pxp_{rep}_{b}")
    st["pxs"] = stats.tile([128, 2048], BF16, tag="pxs", bufs=1, name=f"pxs_{rep}_{b}")

    tps_ring = {}

    def transposes(kt):
        tps = psum.tile([128, C], F32, tag="tstage", bufs=3)
        tps_ring[kt] = tps
        sl = slice(128 * kt, 128 * (kt + 1))
        for m in range(CB):
            nc.tensor.transpose(
                tps[:, 128 * m : 128 * (m + 1)].bitcast(F32R),
                q[:, m, sl],
                ident_r,
            )

    qt_ring = {}

    def evac(kt):
        tps = tps_ring.pop(kt)
        qt = qtpool.tile([128, C], F32R, tag="qt")
        qt_ring[kt] = qt
        if kt % 2 == 0:
            nc.vector.tensor_copy(qt, tps)
        else:
            nc.scalar.copy(qt, tps)

    def mm1(kt):
        qt = qt_ring.pop(kt)
        # kt 0 order (1,2,0,3): eps1/eps2 issue start=True first so their
        # whole-bank has_written clears come first.  eps3 shares bank2 and
        # never issues start=True: its first matmul lands on cleared bits
        # and overwrites.
        order = (1, 2, 0, 3) if kt == 0 else range(CB)
        for m in order:
            st_flag = (kt == 0) and (m != 3)
            nc.tensor.matmul(
                eps[m],
                lhsT=qt[:, 128 * m : 128 * (m + 1)],
                rhs=qt[:, MM1_D0[m] : MM1_D0[m] + MM1_FD[m]],
                start=st_flag,
                stop=(kt == KT - 1),
                skip_group_check=(m == 3),
            )

    def unit(kt):
        # one-slice lookahead: transpose kt+1 before retiring kt, so the PE
        # always has transpose work queued while an evacuation drains.
        # te/mm split lets the caller defer the bank-gated matmuls when
        # interleaving this sample's slices under the previous sample's
        # softmax chain (a deferred mm1 would block the strict PE queue).
        def te():
            if kt == 0:
                transposes(0)
            if kt + 1 < KT:
                transposes(kt + 1)
            evac(kt)

        def mm():
            mm1(kt)
            # staggered pooled-x pieces: by these kts the needed q chunks
            # are resident; ACT sums m=0,1, DVE m=2,3 (split keeps either
            # engine's phase-A queue under its idle budget)
            if 12 <= kt < 14:
                m_, h_ = kt - 12, 0
            elif 27 <= kt < 29:
                m_, h_ = kt - 27, 1
            else:
                m_ = None
            if m_ is not None:
                hsl = slice(2048 * h_, 2048 * (h_ + 1))
                nc.scalar.activation(
                    out=st["pxs"],
                    in_=q[:, m_, hsl].bitcast(F32),
                    func=AF.Copy,
                    accum_out=st["px_part"][:, m_, h_ : h_ + 1],
                )
            if 14 <= kt < 16:
                m_, h_ = kt - 12, 0
            elif 29 <= kt < 31:
                m_, h_ = kt - 27, 1
            else:
                m_ = None
            if m_ is not None:
                hsl = slice(2048 * h_, 2048 * (h_ + 1))
                nc.vector.tensor_reduce(
                    out=st["px_part"][:, m_, h_ : h_ + 1],
                    in_=q[:, m_, hsl].bitcast(F32),
                    axis=AX.X, op=ALU.add,
                )

        return {"te": te, "mm": mm}

    return [unit(kt) for kt in range(KT)]


def _emit_phase_B(tc, st, consts, gpool, stats, psum, rep, b):
    """Assemble full energy rows (upper from PSUM via ACT, lower mirrored via
    PE+DVE), row-min from the PSUM pieces, exp in place (ACT, accum S)."""
    nc = tc.nc
    ident, ident_r, w1T, w2T, b1_t, b2h_t, weight_prep = consts
    eps = st["eps"]
    en = gpool.tile([128, CB, C], F32, tag="en")
    G = gpool.tile([128, CB, C], F32, tag="G")
    st["G"] = G
    nmin_up = stats.tile([128, CB], F32, tag="nmu")
    nmin = stats.tile([128, CB], F32, tag="nmin")
    # row-min of the PSUM-resident upper parts can start as soon as mm1 stops
    for m in range(CB):
        nc.vector.tensor_reduce(
            out=nmin_up[:, m : m + 1], in_=eps[m], axis=AX.X, op=ALU.min
        )
    # upper energy into SBUF (ACT frees the eps banks for the next sample)
    nc.scalar.copy(en[:, 0, :], eps[0])
    nc.scalar.copy(en[:, 1, 128:], eps[1])
    nc.scalar.copy(en[:, 2, 256:], eps[2])
    nc.scalar.copy(en[:, 3, 256:], eps[3])
    nmin_lo = stats.tile([128, CB], F32, tag="nml")
    for m in range(1, CB):
        nmir = 128 * m if m < 3 else 256  # (3,2) already present via eps3
        tpm = psum.tile([128, C], F32, tag="tstage", bufs=3)
        for j in range(nmir // 128):
            nc.tensor.transpose(
                tpm[:, 128 * j : 128 * (j + 1)],
                en[:, j, 128 * m : 128 * (m + 1)],
                ident,
            )
        nc.vector.tensor_reduce(
            out=nmin_lo[:, m : m + 1], in_=tpm[:, :nmir], axis=AX.X, op=ALU.min
        )
        nc.vector.tensor_copy(en[:, m, :nmir], tpm[:, :nmir])
    nc.vector.tensor_copy(nmin[:, 0:1], nmin_up[:, 0:1])
    nc.vector.tensor_tensor(
        out=nmin[:, 1:], in0=nmin_up[:, 1:], in1=nmin_lo[:, 1:], op=ALU.min
    )
    S = stats.tile([128, CB], F32, tag="S")
    st["S"] = S
    for m in range(CB):
        nc.scalar.activation(
            out=G[:, m, :],
            in_=en[:, m, :],
            func=AF.Exp,
            bias=nmin[:, m : m + 1],
            scale=-1.0,
            accum_out=S[:, m : m + 1],
        )


def _emit_pooled(tc, st, stats, rep, b):
    """combine the staggered pooled-x partials and scale by 1/N."""
    nc = tc.nc
    px_raw = stats.tile([128, CB], F32, tag="pxr")
    nc.vector.tensor_reduce(
        out=px_raw, in_=st["px_part"], axis=AX.X, op=ALU.add
    )
    px_mean = stats.tile([128, CB], F32, tag="px")
    st["px_mean"] = px_mean
    nc.scalar.mul(px_mean, px_raw, 1.0 / N)


def _emit_phase_C(tc, st, consts, gtpool, stats, psum, rep, b):
    """G^T staging (k-major, one PSUM bank at a time) + SE gate."""
    nc = tc.nc
    ident, ident_r, w1T, w2T, b1_t, b2h_t, weight_prep = consts
    G = st["G"]
    S = st["S"]
    px_mean = st["px_mean"]
    recipS = stats.tile([128, CB], F32, tag="rS")
    nc.vector.reciprocal(recipS, S)
    GT = gtpool.tile([128, CB, C], F32R, tag="GT")
    st["GT"] = GT
    for k in range(CB):
        gst = psum.tile([128, C], F32, tag="bank", bufs=5, name=f"g_{rep}_{b}_{k}")
        for m in range(CB):
            nc.tensor.transpose(
                gst[:, 128 * m : 128 * (m + 1)],
                G[:, m, 128 * k : 128 * (k + 1)],
                ident,
            )
        nc.vector.tensor_copy(GT[:, k, :], gst)

    # pooled_out = (G @ px_mean) / S
    ps_po = psum.tile([128, CB], F32, tag="tstage", bufs=3)
    for m in range(CB):
        for k in range(CB):
            nc.tensor.matmul(
                ps_po[:, m : m + 1],
                lhsT=GT[:, k, 128 * m : 128 * (m + 1)].bitcast(F32),
                rhs=px_mean[:, k : k + 1],
                start=(k == 0),
                stop=(k == CB - 1),
            )
    po_mean = stats.tile([128, CB], F32, tag="po")
    for m in range(CB):
        nc.scalar.activation(
            po_mean[:, m : m + 1], ps_po[:, m : m + 1], AF.Copy,
            scale=recipS[:, m : m + 1],
        )

    # SE gate: h = relu(w1@pooled+b1); se = sigmoid(w2@h+b2)
    ps_h = psum.tile([64, 1], F32, tag="tstage", bufs=3)
    for k in range(8):
        rhs = px_mean[:, k : k + 1] if k < 4 else po_mean[:, k - 4 : k - 3]
        nc.tensor.matmul(
            ps_h, lhsT=w1T[:, k, :], rhs=rhs, start=(k == 0), stop=(k == 7),
        )
    h_sb = stats.tile([64, 1], F32, tag="h")
    nc.scalar.activation(h_sb, ps_h, AF.Relu, bias=b1_t)

    ps_se = psum.tile([128, CB], F32, tag="tstage", bufs=3)
    for m in range(CB):
        nc.tensor.matmul(
            ps_se[:, m : m + 1], lhsT=w2T[:, m, :], rhs=h_sb,
            start=True, stop=True,
        )
    # sigmoid(z) = 0.5*tanh(z/2) + 0.5 -- tanh shares exp's ACT table set,
    # avoiding two ~2.7us table reloads per sample.
    th = stats.tile([128, CB], F32, tag="th")
    for m in range(CB):
        nc.scalar.activation(
            th[:, m : m + 1], ps_se[:, m : m + 1], AF.Tanh,
            bias=b2h_t[:, m : m + 1], scale=0.5,
        )
    se = stats.tile([128, CB], F32, tag="se")
    st["se"] = se
    nc.vector.tensor_scalar(
        out=se, in0=th, scalar1=0.5, scalar2=0.5, op0=ALU.mult, op1=ALU.add
    )
    beta0 = stats.tile([128, CB], F32, tag="b0")
    beta = stats.tile([128, CB], F32, tag="b1")
    st["beta"] = beta
    nc.vector.tensor_scalar(
        out=beta0, in0=se, scalar1=-1.0, scalar2=1.0, op0=ALU.mult, op1=ALU.add
    )
    nc.vector.tensor_mul(beta, beta0, recipS)


def _phase_D_groups(tc, st, y_d, stats, outp, psum, rep, b):
    """out = G @ q in 3/3/2-chunk PSUM bank groups per row block; the fused
    blend y = se*x + beta*(G@q) is evacuated per chunk (ACT beta-scale to
    bf16, DVE scalar_tensor_tensor), and y streams out per half row."""
    nc = tc.nc
    q = st["q"]
    groups = []

    def group(m, js, fin):
        def emit():
            GT = st["GT"]
            se = st["se"]
            beta = st["beta"]
            banks = [
                psum.tile([128, 512], F32, tag="bank", bufs=5,
                          name=f"po_{rep}_{b}_{m}_{j}")
                for j in js
            ]
            for k in range(CB):
                for bi, j in enumerate(js):
                    nc.tensor.matmul(
                        banks[bi],
                        lhsT=GT[:, k, 128 * m : 128 * (m + 1)],
                        rhs=q[:, k, 512 * j : 512 * (j + 1)],
                        start=(k == 0),
                        stop=(k == CB - 1),
                    )
            for bi, j in enumerate(js):
                nsl = slice(512 * j, 512 * (j + 1))
                ob = outp.tile([128, 512], BF16, tag="ob", bufs=4)
                nc.scalar.activation(
                    ob, banks[bi], AF.Copy, scale=beta[:, m : m + 1],
                )
                nc.vector.scalar_tensor_tensor(
                    out=fin[:, nsl],
                    in0=q[:, m, nsl].bitcast(F32),
                    scalar=se[:, m : m + 1],
                    in1=ob,
                    op0=ALU.mult,
                    op1=ALU.add,
                )
            # stream out per half row as soon as its chunks are blended;
            # the very last row block goes out per quarter to shorten the
            # kernel tail
            last = b == B_PER_CORE - 1 and m == CB - 1
            if js[-1] == 3:
                nc.sync.dma_start(
                    out=y_d[b, 128 * m : 128 * (m + 1), 0:2048],
                    in_=fin[:, 0:2048],
                )
            elif last and js[-1] == 5:
                nc.sync.dma_start(
                    out=y_d[b, 128 * m : 128 * (m + 1), 2048:3072],
                    in_=fin[:, 2048:3072],
                )
            elif last and js[-1] == 7:
                nc.sync.dma_start(
                    out=y_d[b, 128 * m : 128 * (m + 1), 3072:4096],
                    in_=fin[:, 3072:4096],
                )
            elif js[-1] == 7:
                nc.sync.dma_start(
                    out=y_d[b, 128 * m : 128 * (m + 1), 2048:4096],
                    in_=fin[:, 2048:4096],
                )
        return emit

    for m in range(CB):
        fin = outp.tile([128, N], BF16, tag="fin", name=f"fin_{rep}_{b}_{m}")
        for js in ((0, 1), (2, 3), (4, 5), (6, 7)):
            groups.append(group(m, js, fin))
    return groups


_NC_CACHE = None


def _get_program():
    global _NC_CACHE
    if _NC_CACHE is None:
        _NC_CACHE = _build_program()
    return _NC_CACHE


def kernel(x, w1, b1, w2, b2, _trace=False):
    x = np.ascontiguousarray(x, dtype=np.float32)
    B, Cc, H, W = x.shape
    assert (B, Cc, H * W) == (B_TOTAL, C, N)
    xr = x.reshape(B, Cc, H * W)
    in_maps = []
    for i in range(N_CORES):
        in_maps.append(
            {
                "x": np.ascontiguousarray(xr[B_PER_CORE * i : B_PER_CORE * (i + 1)]),
                "w1": np.ascontiguousarray(w1, dtype=np.float32),
                "b1": np.ascontiguousarray(b1, dtype=np.float32).reshape(64, 1),
                "w2": np.ascontiguousarray(w2, dtype=np.float32),
                "b2": np.ascontiguousarray(b2, dtype=np.float32).reshape(C, 1),
            }
        )
    nc = _get_program()
    res = run_bass_kernel_spmd(nc, in_maps, list(range(N_CORES)), trace=_trace)
    y = np.concatenate(
        [np.asarray(res.results[i]["y"]).astype(np.float32) for i in range(N_CORES)],
        axis=0,
    )
    out = y.reshape(B, Cc, H, W)
    if _trace:
        return out, res
    return out
